# revision 8
# baseline (speedup 1.0000x reference)
"""Trainium2 Bass kernel for nn_DelayExpansionLayer (histogram_binning).

Computation: per-channel mean of layer_output [64,256,56,56] over (B,H,W),
round to 1e-6, nearest-key lookup in a sorted 1024-entry table, max over
channels, scale by (in_ch*out_ch)/512, broadcast to (56,56).

Strategy (data-parallel over batch, 8 NeuronCores):
  - Each core gets 8 batches = [8,256,56,56] (25.7 MB) and computes
    per-channel partial sums [256] on-device (DMA-bound reduction).
  - Host combines the 8 partial-sum vectors (the tiny [C] all-reduce),
    then does the O(C+K) lookup/max/broadcast epilogue.

Per-core device kernel (raw bass, manual semaphores):
  input  x [8, 128, 2, 3136] f32  (batch, partition, chan-pair, spatial).
  The HW DGE sprays a DMA's outermost (partition) dim over the 16 SDMA
  engines starting at engine 0, so a 15-partition-wide DMA never touches
  engine 15 (E79) -- the engine that also runs the dynamic queues'
  bookkeeping and moves bytes ~20% slower, pacing any 128-wide stream.
  Queue order: b0-b4 full 3.2MB 128-wide DMAs (all 16 engines), then
  b6,b7 as 8x 15-wide full-line DMAs each (engines 0-14, 25KB packets,
  few descriptors -- a 15-wide free-split stream starves on the ~0.8us
  per-dma_start sequencer cost), then b5 last, 128-wide, free-tapered
  (j0, j1[0:1568], j1[1568:]) so the final reduces are short.  The 8-wide
  partition remainders for b6/b7 and the single [128,2,10] stats out-DMA
  go on the scalar engine's separate HW queue (empty FIFO).  Reduction is
  split across DVE (tensor_reduce) and ACT (activation-Copy accum).
  Channel c = 2*p + j; host sums j0 stats cols 0..7, j1 cols 0..8.
"""

import sys
import types

import numpy as np

N_CORES = 8
B_FULL, C, H, W = 64, 256, 56, 56
HW = H * W
B_LOCAL = B_FULL // N_CORES
SCALE_DENOM = 32 * 16

# Set by a test harness to enable NTFF tracing of the SPMD run.
TRACE = False
TRACE_TMPDIR = None
LAST_RESULTS = None

_CACHE = {}

# 15-wide partition ranges (engines 0-14) + the 8-wide remainder (engines
# 0-7, issued from the scalar queue).
P15 = [(p, p + 15) for p in range(0, 120, 15)]
PREM = (120, 128)


def _ensure_axon_hooks_shim():
    """bass_utils' axon trace path imports antenv.axon_hooks; provide a
    no-op shim when the environment's antenv package lacks it."""
    try:
        import antenv.axon_hooks  # noqa: F401
        return
    except ImportError:
        pass

    mod = types.ModuleType("antenv.axon_hooks")
    _hook = [None]
    mod.set_axon_ntff_profile_hook = lambda h: _hook.__setitem__(0, h)
    mod.get_axon_ntff_profile_hook = lambda: _hook[0]
    sys.modules["antenv.axon_hooks"] = mod
    try:
        import antenv

        antenv.axon_hooks = mod
    except ImportError:
        pass


def _build():
    if "nc" in _CACHE:
        return _CACHE["nc"]
    import concourse.bass as bass
    from concourse import mybir

    nc = bass.Bass(
        "TRN2",
        target_bir_lowering=False,
        debug=False,
        enable_asserts=False,
        num_devices=N_CORES,
    )
    f32 = mybir.dt.float32
    x = nc.dram_tensor("x", [B_LOCAL, 128, 2, HW], f32, kind="ExternalInput").ap()
    out = nc.dram_tensor("out", [128, 2, 10], f32, kind="ExternalOutput").ap()

    # 7 batch slots: b0-b3 -> s0-s3, b4 -> s4, b6 -> s5, b7 -> s6,
    # b5 -> s0 (reused after b0's DVE reduce).
    slots = [
        nc.alloc_sbuf_tensor(f"slot{i}", [128, 2, HW], f32).ap() for i in range(7)
    ]
    stats = nc.alloc_sbuf_tensor("stats", [128, 2, 10], f32).ap()

    with (
        nc.Block(no_gpsimd_drain=True) as block,
        nc.semaphore("ds0") as ds0,
        nc.semaphore("ds1") as ds1,
        nc.semaphore("ds2") as ds2,
        nc.semaphore("ds3") as ds3,
        nc.semaphore("ds4") as ds4,
        nc.semaphore("ds6") as ds6,
        nc.semaphore("ds7") as ds7,
        nc.semaphore("vd") as vd,
        nc.semaphore("ad") as ad,
        nc.semaphore("od") as od,
    ):
        @block.sync
        def _(sync: bass.BassEngine):
            # b0-b4 full-width, no deps
            for b, slot, sem in (
                (0, slots[0], ds0),
                (1, slots[1], ds1),
                (2, slots[2], ds2),
                (3, slots[3], ds3),
                (4, slots[4], ds4),
            ):
                sync.dma_start(out=slot[:], in_=x[b]).then_inc(sem, 16)
            # b6, b7: 15-wide full-line (both j), engines 0-14 only; the
            # 8-wide partition remainder comes from the scalar queue.
            for b, slot, sem in ((6, slots[5], ds6), (7, slots[6], ds7)):
                for p0, p1 in P15:
                    sync.dma_start(
                        out=slot[p0:p1, :, :], in_=x[b, p0:p1, :, :]
                    ).then_inc(sem, 16)
            # b5 last, 128-wide, free-tapered into s0 (needs b0's reduce)
            sync.wait_ge(vd, 1)
            sync.dma_start(out=slots[0][:, 0, :], in_=x[5, :, 0, :]).then_inc(
                ds0, 16
            )
            sync.dma_start(
                out=slots[0][:, 1, 0:1568], in_=x[5, :, 1, 0:1568]
            ).then_inc(ds0, 16)
            sync.dma_start(
                out=slots[0][:, 1, 1568:HW], in_=x[5, :, 1, 1568:HW]
            ).then_inc(ds0, 16)
            sync.wait_ge(od, 16)

        @block.vector
        def _(vector: bass.BassEngine):
            # whole-slot reduces: b0, b2, b6 (ds6 counts exactly its 9
            # DMAs: 8 sync 15-wide + 1 scalar remainder), b7
            for b, slot, sem, thr in (
                (0, slots[0], ds0, 16),
                (2, slots[2], ds2, 16),
                (6, slots[5], ds6, 144),
                (7, slots[6], ds7, 144),
            ):
                vector.wait_ge(sem, thr)
                vector.reduce_sum(
                    stats[:, :, b : b + 1], slot[:], axis=mybir.AxisListType.X
                ).then_inc(vd, 1)
            # b5 j1[0:1568] -> j1 col 5
            vector.wait_ge(ds0, 48)
            vector.reduce_sum(
                stats[:, 1, 5:6],
                slots[0][:, 1, 0:1568],
                axis=mybir.AxisListType.X,
            ).then_inc(vd, 1)

        @block.scalar
        def _(scalar: bass.BassEngine):
            # 8-wide partition remainders for b6/b7, enqueued up front on
            # the (empty) scalar HW queue -- fresh slots, no deps.
            p0, p1 = PREM
            for b, slot, sem in ((6, slots[5], ds6), (7, slots[6], ds7)):
                scalar.dma_start(
                    out=slot[p0:p1, :, :], in_=x[b, p0:p1, :, :]
                ).then_inc(sem, 16)

            # ACT accum pair-reduces: b1, b3, b4
            for b, slot, sem in (
                (1, slots[1], ds1),
                (3, slots[3], ds3),
                (4, slots[4], ds4),
            ):
                scalar.wait_ge(sem, 16)
                for j in range(2):
                    ins = scalar.activation(
                        slot[:, j, :],
                        slot[:, j, :],
                        mybir.ActivationFunctionType.Copy,
                        accum_out=stats[:, j, b : b + 1],
                    )
                    if j == 1:
                        ins.then_inc(ad, 1)
            # b5 j0 -> j0 col 5; b5 j1[1568:] -> j1 col 8
            scalar.wait_ge(ds0, 32)
            scalar.activation(
                slots[0][:, 0, :],
                slots[0][:, 0, :],
                mybir.ActivationFunctionType.Copy,
                accum_out=stats[:, 0, 5:6],
            ).then_inc(ad, 1)
            scalar.wait_ge(ds0, 64)
            scalar.activation(
                slots[0][:, 1, 1568:HW],
                slots[0][:, 1, 1568:HW],
                mybir.ActivationFunctionType.Copy,
                accum_out=stats[:, 1, 8:9],
            ).then_inc(ad, 1)
            # single out-DMA on the scalar queue (bypasses input FIFO).
            # ad>=5 orders it after ACT's last accumulator writeback (the
            # inc fires post-writeback); vd>=5 after DVE's last reduce.
            scalar.wait_ge(ad, 5)
            scalar.wait_ge(vd, 5)
            scalar.dma_start(out=out[:], in_=stats[:]).then_inc(od, 16)

    _CACHE["nc"] = nc
    return nc


def kernel(layer_output, delay_keys, delay_values, in_channels, out_channels):
    global LAST_RESULTS
    _ensure_axon_hooks_shim()
    from concourse.bass_utils import run_bass_kernel_spmd

    x = np.ascontiguousarray(np.asarray(layer_output, dtype=np.float32))
    assert x.shape == (B_FULL, C, H, W), x.shape
    # shard over batch; view channels as (partition, pair): c = 2*p + j
    xr = x.reshape(N_CORES, B_LOCAL, 128, 2, HW)
    in_maps = [{"x": xr[k]} for k in range(N_CORES)]

    nc = _build()
    kwargs = {}
    if TRACE:
        kwargs.update(trace=True, tmpdir=TRACE_TMPDIR)
    res = run_bass_kernel_spmd(nc, in_maps, core_ids=list(range(N_CORES)), **kwargs)
    LAST_RESULTS = res

    # tiny [C] all-reduce of the per-core partial sums
    parts = np.stack(
        [res.results[k]["out"] for k in range(N_CORES)]
    )  # [8, 128, 2, 10]; j=0 valid cols 0..7, j=1 valid cols 0..8
    s0 = parts[:, :, 0, 0:8].sum(axis=(0, 2), dtype=np.float32)
    s1 = parts[:, :, 1, 0:9].sum(axis=(0, 2), dtype=np.float32)
    sums = np.stack([s0, s1], axis=1).reshape(C)  # c = 2p+j
    means = sums / np.float32(B_FULL * HW)
    means = np.round(means * np.float32(1e6)) / np.float32(1e6)

    keys = np.asarray(delay_keys, dtype=np.float32)
    values = np.asarray(delay_values, dtype=np.float32)
    K = keys.shape[0]
    idx = np.searchsorted(keys, means)
    lo = np.clip(idx - 1, 0, K - 1)
    hi = np.clip(idx, 0, K - 1)
    pick_hi = np.abs(keys[hi] - means) < np.abs(keys[lo] - means)
    nearest = np.where(pick_hi, hi, lo)
    merged = np.float32(values[nearest].max())

    scale = np.float32(
        (int(np.asarray(in_channels)) * int(np.asarray(out_channels))) / SCALE_DENOM
    )
    return np.full((H, W), merged, dtype=np.float32) * scale


# revision 9
# speedup vs baseline: 1.1014x; 1.1014x over previous
"""Trainium2 Bass kernel for nn_DelayExpansionLayer (histogram_binning).

Computation: per-channel mean of layer_output [64,256,56,56] over (B,H,W),
round to 1e-6, nearest-key lookup in a sorted 1024-entry table, max over
channels, scale by (in_ch*out_ch)/512, broadcast to (56,56).

Strategy (data-parallel over batch, 8 NeuronCores):
  - Each core gets 8 batches = [8,256,56,56] (25.7 MB) and computes
    per-channel partial sums [256] on-device (DMA-bound reduction).
  - Host combines the 8 partial-sum vectors (the tiny [C] all-reduce),
    then does the O(C+K) lookup/max/broadcast epilogue.

Per-core device kernel (raw bass, manual semaphores):
  input  x [8, 128, 2, 3136] f32  (batch, partition, chan-pair, spatial).
  Two facts drive the layout (measured from NTFF traces):
    1. The HW DGE sprays a DMA's outermost (partition) dim over the 16
       SDMA engines starting at engine 0, so a 15-partition-wide DMA
       never touches engine 15 (E79) -- the engine that also runs the
       dynamic queues' bookkeeping and moves bytes ~20% slower.
    2. A queue executes its DMA instructions ~serially (slowest engine's
       share + ~1-2us spray/completion overhead each), so 15-wide
       split streams only pay off when spread over BOTH hardware queues.
  Queue layout: Q-sync: b0, b2, b4 full 128-wide (all 16 engines), b6 as
  8x 15-wide full-line DMAs (engines 0-14), then b5 last, 128-wide,
  free-tapered (j0, j1[0:1568], j1[1568:]) so the final reduces are
  short.  Q-scalar: b1, b3 full-wide, b7 as 8x 15-wide, the two 8-wide
  partition remainders, and the single [128,2,10] stats out-DMA.  E79
  carries only the six full-wide batches (~55us) while engines 0-14
  carry ~61-63us; the two queues overlap so neither's instruction
  serialization paces the stream.  Reduction is split across DVE
  (tensor_reduce) and ACT (activation-Copy accum).
  Channel c = 2*p + j; host sums j0 stats cols 0..7, j1 cols 0..8.
"""

import sys
import types

import numpy as np

N_CORES = 8
B_FULL, C, H, W = 64, 256, 56, 56
HW = H * W
B_LOCAL = B_FULL // N_CORES
SCALE_DENOM = 32 * 16

# Set by a test harness to enable NTFF tracing of the SPMD run.
TRACE = False
TRACE_TMPDIR = None
LAST_RESULTS = None

_CACHE = {}

# 15-wide partition ranges (engines 0-14) + the 8-wide remainder (engines
# 0-7, issued from the scalar queue).
P15 = [(p, p + 15) for p in range(0, 120, 15)]
PREM = (120, 128)


def _ensure_axon_hooks_shim():
    """bass_utils' axon trace path imports antenv.axon_hooks; provide a
    no-op shim when the environment's antenv package lacks it."""
    try:
        import antenv.axon_hooks  # noqa: F401
        return
    except ImportError:
        pass

    mod = types.ModuleType("antenv.axon_hooks")
    _hook = [None]
    mod.set_axon_ntff_profile_hook = lambda h: _hook.__setitem__(0, h)
    mod.get_axon_ntff_profile_hook = lambda: _hook[0]
    sys.modules["antenv.axon_hooks"] = mod
    try:
        import antenv

        antenv.axon_hooks = mod
    except ImportError:
        pass


def _build():
    if "nc" in _CACHE:
        return _CACHE["nc"]
    import concourse.bass as bass
    from concourse import mybir

    nc = bass.Bass(
        "TRN2",
        target_bir_lowering=False,
        debug=False,
        enable_asserts=False,
        num_devices=N_CORES,
    )
    f32 = mybir.dt.float32
    x = nc.dram_tensor("x", [B_LOCAL, 128, 2, HW], f32, kind="ExternalInput").ap()
    out = nc.dram_tensor("out", [128, 2, 10], f32, kind="ExternalOutput").ap()

    # 7 batch slots: b0-b4 -> s0-s4, b6 -> s5, b7 -> s6,
    # b5 -> s0 (reused after b0's DVE reduce).
    slots = [
        nc.alloc_sbuf_tensor(f"slot{i}", [128, 2, HW], f32).ap() for i in range(7)
    ]
    stats = nc.alloc_sbuf_tensor("stats", [128, 2, 10], f32).ap()

    with (
        nc.Block(no_gpsimd_drain=True) as block,
        nc.semaphore("ds0") as ds0,
        nc.semaphore("ds1") as ds1,
        nc.semaphore("ds2") as ds2,
        nc.semaphore("ds3") as ds3,
        nc.semaphore("ds4") as ds4,
        nc.semaphore("ds6") as ds6,
        nc.semaphore("ds7") as ds7,
        nc.semaphore("vd") as vd,
        nc.semaphore("ad") as ad,
        nc.semaphore("od") as od,
    ):
        @block.sync
        def _(sync: bass.BassEngine):
            # b0, b2, b4 full-width, no deps
            for b, slot, sem in (
                (0, slots[0], ds0),
                (2, slots[2], ds2),
                (4, slots[4], ds4),
            ):
                sync.dma_start(out=slot[:], in_=x[b]).then_inc(sem, 16)
            # b6: 15-wide full-line (both j), engines 0-14; the 8-wide
            # partition remainder comes from the scalar queue.
            for p0, p1 in P15:
                sync.dma_start(
                    out=slots[5][p0:p1, :, :], in_=x[6, p0:p1, :, :]
                ).then_inc(ds6, 16)
            # b5 last, 128-wide, free-tapered into s0 (needs b0's reduce)
            sync.wait_ge(vd, 1)
            sync.dma_start(out=slots[0][:, 0, :], in_=x[5, :, 0, :]).then_inc(
                ds0, 16
            )
            sync.dma_start(
                out=slots[0][:, 1, 0:1568], in_=x[5, :, 1, 0:1568]
            ).then_inc(ds0, 16)
            sync.dma_start(
                out=slots[0][:, 1, 1568:HW], in_=x[5, :, 1, 1568:HW]
            ).then_inc(ds0, 16)
            sync.wait_ge(od, 16)

        @block.vector
        def _(vector: bass.BassEngine):
            # whole-slot reduces: b0, b2, b4, b6 (ds6 counts exactly its
            # 9 DMAs: 8 sync 15-wide + 1 scalar remainder)
            for b, slot, sem, thr in (
                (0, slots[0], ds0, 16),
                (2, slots[2], ds2, 16),
                (4, slots[4], ds4, 16),
                (6, slots[5], ds6, 144),
            ):
                vector.wait_ge(sem, thr)
                vector.reduce_sum(
                    stats[:, :, b : b + 1], slot[:], axis=mybir.AxisListType.X
                ).then_inc(vd, 1)
            # b5 j1[0:1568] -> j1 col 5
            vector.wait_ge(ds0, 48)
            vector.reduce_sum(
                stats[:, 1, 5:6],
                slots[0][:, 1, 0:1568],
                axis=mybir.AxisListType.X,
            ).then_inc(vd, 1)

        @block.scalar
        def _(scalar: bass.BassEngine):
            # Q-scalar DMAs, enqueued up front (all into fresh slots):
            # b1, b3 full-wide, b7 15-wide, the b6/b7 8-wide remainders.
            for b, slot, sem in ((1, slots[1], ds1), (3, slots[3], ds3)):
                scalar.dma_start(out=slot[:], in_=x[b]).then_inc(sem, 16)
            for p0, p1 in P15:
                scalar.dma_start(
                    out=slots[6][p0:p1, :, :], in_=x[7, p0:p1, :, :]
                ).then_inc(ds7, 16)
            p0, p1 = PREM
            for b, slot, sem in ((6, slots[5], ds6), (7, slots[6], ds7)):
                scalar.dma_start(
                    out=slot[p0:p1, :, :], in_=x[b, p0:p1, :, :]
                ).then_inc(sem, 16)

            # ACT accum pair-reduces: b1, b3, b7
            for b, slot, sem, thr in (
                (1, slots[1], ds1, 16),
                (3, slots[3], ds3, 16),
                (7, slots[6], ds7, 144),
            ):
                scalar.wait_ge(sem, thr)
                for j in range(2):
                    ins = scalar.activation(
                        slot[:, j, :],
                        slot[:, j, :],
                        mybir.ActivationFunctionType.Copy,
                        accum_out=stats[:, j, b : b + 1],
                    )
                    if j == 1:
                        ins.then_inc(ad, 1)
            # b5 j0 -> j0 col 5; b5 j1[1568:] -> j1 col 8
            scalar.wait_ge(ds0, 32)
            scalar.activation(
                slots[0][:, 0, :],
                slots[0][:, 0, :],
                mybir.ActivationFunctionType.Copy,
                accum_out=stats[:, 0, 5:6],
            ).then_inc(ad, 1)
            scalar.wait_ge(ds0, 64)
            scalar.activation(
                slots[0][:, 1, 1568:HW],
                slots[0][:, 1, 1568:HW],
                mybir.ActivationFunctionType.Copy,
                accum_out=stats[:, 1, 8:9],
            ).then_inc(ad, 1)
            # single out-DMA on the scalar queue (its input stream work is
            # long done by now).  ad>=5 orders it after ACT's last
            # accumulator writeback; vd>=5 after DVE's last reduce.
            scalar.wait_ge(ad, 5)
            scalar.wait_ge(vd, 5)
            scalar.dma_start(out=out[:], in_=stats[:]).then_inc(od, 16)

    _CACHE["nc"] = nc
    return nc


def kernel(layer_output, delay_keys, delay_values, in_channels, out_channels):
    global LAST_RESULTS
    _ensure_axon_hooks_shim()
    from concourse.bass_utils import run_bass_kernel_spmd

    x = np.ascontiguousarray(np.asarray(layer_output, dtype=np.float32))
    assert x.shape == (B_FULL, C, H, W), x.shape
    # shard over batch; view channels as (partition, pair): c = 2*p + j
    xr = x.reshape(N_CORES, B_LOCAL, 128, 2, HW)
    in_maps = [{"x": xr[k]} for k in range(N_CORES)]

    nc = _build()
    kwargs = {}
    if TRACE:
        kwargs.update(trace=True, tmpdir=TRACE_TMPDIR)
    res = run_bass_kernel_spmd(nc, in_maps, core_ids=list(range(N_CORES)), **kwargs)
    LAST_RESULTS = res

    # tiny [C] all-reduce of the per-core partial sums
    parts = np.stack(
        [res.results[k]["out"] for k in range(N_CORES)]
    )  # [8, 128, 2, 10]; j=0 valid cols 0..7, j=1 valid cols 0..8
    s0 = parts[:, :, 0, 0:8].sum(axis=(0, 2), dtype=np.float32)
    s1 = parts[:, :, 1, 0:9].sum(axis=(0, 2), dtype=np.float32)
    sums = np.stack([s0, s1], axis=1).reshape(C)  # c = 2p+j
    means = sums / np.float32(B_FULL * HW)
    means = np.round(means * np.float32(1e6)) / np.float32(1e6)

    keys = np.asarray(delay_keys, dtype=np.float32)
    values = np.asarray(delay_values, dtype=np.float32)
    K = keys.shape[0]
    idx = np.searchsorted(keys, means)
    lo = np.clip(idx - 1, 0, K - 1)
    hi = np.clip(idx, 0, K - 1)
    pick_hi = np.abs(keys[hi] - means) < np.abs(keys[lo] - means)
    nearest = np.where(pick_hi, hi, lo)
    merged = np.float32(values[nearest].max())

    scale = np.float32(
        (int(np.asarray(in_channels)) * int(np.asarray(out_channels))) / SCALE_DENOM
    )
    return np.full((H, W), merged, dtype=np.float32) * scale


# revision 10
# speedup vs baseline: 1.4325x; 1.3006x over previous
"""Trainium2 Bass kernel for nn_DelayExpansionLayer (histogram_binning).

Computation: per-channel mean of layer_output [64,256,56,56] over (B,H,W),
round to 1e-6, nearest-key lookup in a sorted 1024-entry table, max over
channels, scale by (in_ch*out_ch)/512, broadcast to (56,56).

Strategy (data-parallel over batch, 8 NeuronCores):
  - Each core gets 8 batches = [8,256,56,56] (25.7 MB) and computes
    per-channel partial sums [256] on-device (DMA-bound reduction).
  - Host combines the 8 partial-sum vectors (the tiny [C] all-reduce),
    then does the O(C+K) lookup/max/broadcast epilogue.

Per-core device kernel (raw bass, manual semaphores, ~75us = HBM-line-rate
bound; stream alone is ~61us at ~421 GB/s):
  input  x [8, 128, 2, 3136] f32  (batch, partition, chan-pair, spatial);
  batches 0-6 load as full 3.2MB contiguous DMAs (25KB/partition packets --
  smaller packets trigger a ~20% slowdown on SDMA engine 15 that stretches
  the stream), batch 7 is tapered (j0, then j1 as 1568/784/784) so the last
  reduce lands ~1us after the last byte. Reduction is split across DVE
  (tensor_reduce, batches 0/2/4/6 + one tail chunk) and ACT (activation-
  Copy with accum_out, batches 1/3/5 + three tail chunks) so neither
  engine paces the DMA stream. Partial sums stats[128, 2, 10] go out in
  two DMAs (early cols 0-5, final cols 6-9); channel c = 2*p + j.
"""

import sys
import types

import numpy as np

N_CORES = 8
B_FULL, C, H, W = 64, 256, 56, 56
HW = H * W
B_LOCAL = B_FULL // N_CORES
SCALE_DENOM = 32 * 16

# Set by a test harness to enable NTFF tracing of the SPMD run.
TRACE = False
TRACE_TMPDIR = None
LAST_RESULTS = None

_CACHE = {}


def _ensure_axon_hooks_shim():
    """bass_utils' axon trace path imports antenv.axon_hooks; provide a
    no-op shim when the environment's antenv package lacks it."""
    try:
        import antenv.axon_hooks  # noqa: F401
        return
    except ImportError:
        pass

    mod = types.ModuleType("antenv.axon_hooks")
    _hook = [None]
    mod.set_axon_ntff_profile_hook = lambda h: _hook.__setitem__(0, h)
    mod.get_axon_ntff_profile_hook = lambda: _hook[0]
    sys.modules["antenv.axon_hooks"] = mod
    try:
        import antenv

        antenv.axon_hooks = mod
    except ImportError:
        pass


def _build():
    """Raw-bass (no TileContext) SPMD kernel with manual semaphores.

    Per core: 11 input DMAs (7 full 3.2MB batch tiles + 4 tapered tail
    chunks), reduction split across DVE (tensor_reduce) and ACT
    (activation-Copy accum), partial sums [128,2,10] DMAed out in two
    pieces. Manual sems avoid Tile's entry/exit barriers (~3us).
    """
    if "nc" in _CACHE:
        return _CACHE["nc"]
    import concourse.bass as bass
    from concourse import mybir

    nc = bass.Bass(
        "TRN2",
        target_bir_lowering=False,
        debug=False,
        enable_asserts=False,
        num_devices=N_CORES,
    )
    f32 = mybir.dt.float32
    x = nc.dram_tensor("x", [B_LOCAL, 128, 2, HW], f32, kind="ExternalInput").ap()
    out = nc.dram_tensor("out", [128, 2, 10], f32, kind="ExternalOutput").ap()

    # SBUF buffers: 4 pair slots (25KB/part) + 4 tail chunks + stats
    slots = [
        nc.alloc_sbuf_tensor(f"slot{i}", [128, 2, HW], f32).ap() for i in range(4)
    ]
    tails = [
        nc.alloc_sbuf_tensor(f"tail{i}", [128, HW], f32).ap() for i in range(4)
    ]
    stats = nc.alloc_sbuf_tensor("stats", [128, 2, 10], f32).ap()

    # tail chunks: (j, s0, s1, engine, stats col)
    TAIL = (
        (0, 0, HW, "a", 7),
        (1, 0, 1568, "v", 7),
        (1, 1568, 2352, "a", 8),
        (1, 2352, HW, "a", 9),
    )

    with (
        nc.Block(no_gpsimd_drain=True) as block,
        nc.semaphore("ds0") as ds0,
        nc.semaphore("ds1") as ds1,
        nc.semaphore("ds2") as ds2,
        nc.semaphore("ds3") as ds3,
        nc.semaphore("dt0") as dt0,
        nc.semaphore("dt1") as dt1,
        nc.semaphore("dt2") as dt2,
        nc.semaphore("dt3") as dt3,
        nc.semaphore("vd") as vd,
        nc.semaphore("ad") as ad,
        nc.semaphore("od") as od,
    ):
        ds = [ds0, ds1, ds2, ds3]
        dt = [dt0, dt1, dt2, dt3]

        @block.sync
        def _(sync: bass.BassEngine):
            # batches 0-3 into slots 0-3, no deps
            for b in range(4):
                sync.dma_start(out=slots[b][:], in_=x[b]).then_inc(ds[b], 16)
            # batch 4 reuses slot 0: needs b0's DVE reduce (vd>=1)
            sync.wait_ge(vd, 1)
            sync.dma_start(out=slots[0][:], in_=x[4]).then_inc(ds[0], 16)
            # batch 5 reuses slot 1: needs b1's ACT pair done (ad>=1)
            sync.wait_ge(ad, 1)
            sync.dma_start(out=slots[1][:], in_=x[5]).then_inc(ds[1], 16)
            # batch 6 reuses slot 2: needs b2's DVE reduce (vd>=2)
            sync.wait_ge(vd, 2)
            sync.dma_start(out=slots[2][:], in_=x[6]).then_inc(ds[2], 16)
            # tail chunks: fresh buffers, no deps
            for i, (j, s0, s1, _e, _k) in enumerate(TAIL):
                w = s1 - s0
                sync.dma_start(
                    out=tails[i][:, 0:w], in_=x[B_LOCAL - 1, :, j, s0:s1]
                ).then_inc(dt[i], 16)
            # early out-DMA for batch columns 0..5 once their reduces done
            sync.wait_ge(vd, 3)
            sync.wait_ge(ad, 3)
            sync.dma_start(out=out[:, :, 0:6], in_=stats[:, :, 0:6]).then_inc(
                od, 16
            )
            # final out-DMA (cols 6..9) from the pre-armed idle sync engine.
            # ad>=6 orders it after the last ACTIVATE's accumulator
            # writeback (the update fires post-writeback); vd>=5 after
            # DVE's tail reduce.
            sync.wait_ge(ad, 6)
            sync.wait_ge(vd, 5)
            sync.dma_start(out=out[:, :, 6:10], in_=stats[:, :, 6:10]).then_inc(
                od, 16
            )
            sync.wait_ge(od, 32)

        @block.vector
        def _(vector: bass.BassEngine):
            # pair reduces: batches 0,2,4,6 -> stats[:,:,b]
            for b, sem, thr in ((0, ds0, 16), (2, ds2, 16), (4, ds0, 32), (6, ds2, 32)):
                vector.wait_ge(sem, thr)
                slot = slots[b % 4]
                vector.reduce_sum(
                    stats[:, :, b : b + 1], slot[:], axis=mybir.AxisListType.X
                ).then_inc(vd, 1)
            # tail chunk 1 (j1 cols 0:1568)
            i, (j, s0, s1, _e, k) = 1, TAIL[1]
            vector.wait_ge(dt[i], 16)
            vector.reduce_sum(
                stats[:, j, k : k + 1],
                tails[i][:, 0 : s1 - s0],
                axis=mybir.AxisListType.X,
            ).then_inc(vd, 1)

        @block.scalar
        def _(scalar: bass.BassEngine):
            # ACT batches 1,3,5: two activation-accum ops each
            for b, sem, thr in ((1, ds1, 16), (3, ds3, 16), (5, ds1, 32)):
                scalar.wait_ge(sem, thr)
                slot = slots[b % 4]
                for j in range(2):
                    ins = scalar.activation(
                        slot[:, j, :],
                        slot[:, j, :],
                        mybir.ActivationFunctionType.Copy,
                        accum_out=stats[:, j, b : b + 1],
                    )
                    if j == 1:
                        ins.then_inc(ad, 1)
            # tail chunks 0, 2, 3
            for i in (0, 2, 3):
                j, s0, s1, _e, k = TAIL[i]
                scalar.wait_ge(dt[i], 16)
                scalar.activation(
                    tails[i][:, 0 : s1 - s0],
                    tails[i][:, 0 : s1 - s0],
                    mybir.ActivationFunctionType.Copy,
                    accum_out=stats[:, j, k : k + 1],
                ).then_inc(ad, 1)

    _CACHE["nc"] = nc
    return nc


def kernel(layer_output, delay_keys, delay_values, in_channels, out_channels):
    global LAST_RESULTS
    _ensure_axon_hooks_shim()
    from concourse.bass_utils import run_bass_kernel_spmd

    x = np.ascontiguousarray(np.asarray(layer_output, dtype=np.float32))
    assert x.shape == (B_FULL, C, H, W), x.shape
    # shard over batch; view channels as (partition, pair): c = 2*p + j
    xr = x.reshape(N_CORES, B_LOCAL, 128, 2, HW)
    in_maps = [{"x": xr[k]} for k in range(N_CORES)]

    nc = _build()
    kwargs = {}
    if TRACE:
        kwargs.update(trace=True, tmpdir=TRACE_TMPDIR)
    res = run_bass_kernel_spmd(nc, in_maps, core_ids=list(range(N_CORES)), **kwargs)
    LAST_RESULTS = res

    # tiny [C] all-reduce of the per-core partial sums
    parts = np.stack(
        [res.results[k]["out"] for k in range(N_CORES)]
    )  # [8, 128, 2, 10]; j=0 valid cols 0..7, j=1 valid cols 0..9
    s0 = parts[:, :, 0, 0:8].sum(axis=(0, 2), dtype=np.float32)
    s1 = parts[:, :, 1, 0:10].sum(axis=(0, 2), dtype=np.float32)
    sums = np.stack([s0, s1], axis=1).reshape(C)  # c = 2p+j
    means = sums / np.float32(B_FULL * HW)
    means = np.round(means * np.float32(1e6)) / np.float32(1e6)

    keys = np.asarray(delay_keys, dtype=np.float32)
    values = np.asarray(delay_values, dtype=np.float32)
    K = keys.shape[0]
    idx = np.searchsorted(keys, means)
    lo = np.clip(idx - 1, 0, K - 1)
    hi = np.clip(idx, 0, K - 1)
    pick_hi = np.abs(keys[hi] - means) < np.abs(keys[lo] - means)
    nearest = np.where(pick_hi, hi, lo)
    merged = np.float32(values[nearest].max())

    scale = np.float32(
        (int(np.asarray(in_channels)) * int(np.asarray(out_channels))) / SCALE_DENOM
    )
    return np.full((H, W), merged, dtype=np.float32) * scale
